# revision 16
# baseline (speedup 1.0000x reference)
# Self-contained Trainium2 Bass kernel for nn_AttentionBlock (AdaLN + QK-norm
# attention), fp8-DoubleRow edition with per-head pipelining.
#
# Sharding: 8 cores = 4 batches (data parallel) x 2 head-groups of 8 heads
# (tensor parallel).  Each core computes, for its batch b and head group g:
#   xn^T  = (rmsnorm(x_b) * (1+scale) + shift)^T    fp8 e4m3 [dim, n]
#   v     = fp8 DR proj with e4m3-hi + e5m2-lo weight split  -> e4m3
#   per head h (pipelined so Act's exp overlaps the next head's proj):
#     q,k = fp8 DR proj -> staged e4m3 -> QK-rmsnorm -> e4m3 -> d-split DMA
#     E   = exp(logits/16 - ln32) e4m3; denom via fp8 DR ones-matmul
#     o^T = fp8 DR (v^T E) * recip(denom), bf16, kept in SBUF
#   out   = o @ (W_out + I) in bf16  -> bf16 partial, host sums the 2 groups
import numpy as np

B, N, DIM = 4, 2048, 2048
H_TOT, D = 16, 128
HG = 2                # head groups (tensor-parallel)
H = H_TOT // HG       # heads per core = 8
QK = H * D            # 1024 q (or k) columns per core
KC = DIM // 128       # 16 dim chunks
KCP = KC // 2         # 8 dim-chunk pairs (DoubleRow)
TC = N // 128         # 16 token chunks
EPS = 1e-6
NCORES = 8
SW = 32.0             # host fp8 scale on W_qkv
SST = 1.0 / 32.0      # q/k psum staging scale
SQK = 4.0             # q/k post-norm fp8 scale
SV = 4.0              # v fp8 scale
LN32 = float(np.log(32.0))

_COMPILED = None


def _build(stop_after=None):
    import concourse.bass as bass
    import concourse.bacc as bacc
    import concourse.tile as tile
    from concourse import mybir
    from concourse.masks import make_identity

    f32 = mybir.dt.float32
    bf16 = mybir.dt.bfloat16
    f8 = mybir.dt.float8e4
    f8l = mybir.dt.float8e5
    AF = mybir.ActivationFunctionType
    OP = mybir.AluOpType

    nc = bacc.Bacc(
        "TRN2", target_bir_lowering=False, debug=False, num_devices=NCORES
    )

    # ---- DRAM I/O -------------------------------------------------------
    x_b = nc.dram_tensor("x_b", [N, DIM], bf16, kind="ExternalInput").ap()
    mcol_in = nc.dram_tensor("mcol_in", [128, KC], f32, kind="ExternalInput").ap()
    scol_in = nc.dram_tensor("scol_in", [128, KC], f32, kind="ExternalInput").ap()
    qs_in = nc.dram_tensor("qs_in", [128, 2], f32, kind="ExternalInput").ap()
    ks_in = nc.dram_tensor("ks_in", [128, 2], f32, kind="ExternalInput").ap()
    Wqk8 = nc.dram_tensor("Wqk8", [DIM, 2 * QK], f8, kind="ExternalInput").ap()
    Wvhi8 = nc.dram_tensor("Wvhi8", [DIM, QK], f8, kind="ExternalInput").ap()
    Wvlo8 = nc.dram_tensor("Wvlo8", [DIM, QK], f8l, kind="ExternalInput").ap()
    Wout_s = nc.dram_tensor("Wout_s", [QK, DIM], bf16, kind="ExternalInput").ap()
    out_p = nc.dram_tensor("out_p", [N, DIM], bf16, kind="ExternalOutput").ap()

    with tile.TileContext(nc) as tc:
        _emit(nc, tc, bass, mybir, tile, make_identity, f32, bf16, f8, f8l,
              AF, OP, x_b, mcol_in, scol_in, qs_in, ks_in, Wqk8, Wvhi8,
              Wvlo8, Wout_s, out_p, stop_after)
    nc.compile()
    return nc


def _emit(nc, tc, bass, mybir, tile, make_identity, f32, bf16, f8, f8l,
          AF, OP, x_b, mcol_in, scol_in, qs_in, ks_in, Wqk8, Wvhi8,
          Wvlo8, Wout_s, out_p, stop_after=None):
    from contextlib import ExitStack

    ts = bass.ts
    DR = mybir.MatmulPerfMode.DoubleRow

    wqk = Wqk8.rearrange("(c p) n -> p c n", p=128)     # [128, KC, 2*QK]
    wvh = Wvhi8.rearrange("(c p) n -> p c n", p=128)    # [128, KC, QK]
    wvl = Wvlo8.rearrange("(c p) n -> p c n", p=128)
    wout = Wout_s.rearrange("(c p) n -> p c n", p=128)  # [128, H, DIM]

    with ExitStack() as ctx:
        consts = ctx.enter_context(tc.tile_pool(name="consts", bufs=1))

        ident = consts.tile([128, 128], bf16)
        make_identity(nc, ident)
        ones_col = consts.tile([128, 1], bf16)
        nc.vector.memset(ones_col, 1.0)
        ones8b = consts.tile([128, 2, 128], f8)   # DR denominator lhsT
        nc.vector.memset(ones8b, 4.0)             # folds E/32 and vS=4v scales
        onesq = consts.tile([128, 128], f8)       # colsum-broadcast lhsT
        nc.vector.memset(onesq, 1.0)

        mcol = consts.tile([128, KC], f32)          # g_norm*(1+scale) columns
        nc.sync.dma_start(out=mcol, in_=mcol_in)
        scol = consts.tile([128, KC], f32)          # shift columns
        nc.sync.dma_start(out=scol, in_=scol_in)
        qsb = consts.tile([128, 2], f32)            # q norm Sqrt scale|bias
        nc.sync.dma_start(out=qsb, in_=qs_in)
        ksb = consts.tile([128, 2], f32)            # k norm Sqrt scale|bias
        nc.sync.dma_start(out=ksb, in_=ks_in)
        eps128 = consts.tile([128, 1], f32)
        nc.vector.memset(eps128, EPS)
        eps1 = consts.tile([1, 1], f32)
        nc.vector.memset(eps1, EPS)
        ln32n = consts.tile([128, 1], f32)
        nc.vector.memset(ln32n, -LN32)

        # persistent across phases
        vS = consts.tile([128, TC, QK], f8)         # v * SV, token-major
        qT8s = consts.tile([64, 2, H, N], f8)       # d-split normalized q
        kT8s = consts.tile([64, 2, H, N], f8)
        oT = consts.tile([128, H, N], bf16)         # o^T, stays in SBUF
        if stop_after == 'ph0':
            return

        with ExitStack() as ph2stack:
            xnp = ph2stack.enter_context(tc.tile_pool(name="xnp", bufs=1))
            xnT8 = xnp.tile([128, KC, N], f8)
            stg = ph2stack.enter_context(tc.tile_pool(name="stg", bufs=2))
            ph2w = ph2stack.enter_context(tc.tile_pool(name="ph2w", bufs=3))
            # shared psum for qk-proj ps / v-proj ps / attention po
            shps = ph2stack.enter_context(
                tc.tile_pool(name="shps", bufs=1, space="PSUM"))

            # ---------- phase 1 + v-projection (before the head loop) ----
            with ExitStack() as innerA:
                ph1 = innerA.enter_context(tc.tile_pool(name="ph1", bufs=2))
                ph1s = innerA.enter_context(tc.tile_pool(name="ph1s", bufs=4))
                ph1sq = innerA.enter_context(tc.tile_pool(name="ph1sq", bufs=1))
                ph2v = innerA.enter_context(tc.tile_pool(name="ph2v", bufs=1))
                ph1ps = innerA.enter_context(
                    tc.tile_pool(name="ph1ps", bufs=2, space="PSUM"))

                def emit_ph1_group(tg):
                    for half in range(2):
                        xts, dgs = [], []
                        for tt in range(2):
                            t = tg * 4 + half * 2 + tt
                            xt = ph1.tile([128, DIM], bf16, tag="xt",
                                          name=f"xt{t}")
                            nc.sync.dma_start(out=xt, in_=x_b[ts(t, 128), :])
                            sq = ph1sq.tile([128, DIM], bf16, tag="sq",
                                            name=f"sq{t}")
                            ssq = ph1s.tile([128, 1], f32, tag="ssq",
                                            name=f"ssq{t}")
                            nc.scalar.activation(sq, xt, AF.Square,
                                                 accum_out=ssq)
                            rin = ph1s.tile([128, 1], f32, tag="rin",
                                            name=f"ri{t}")
                            nc.scalar.activation(rin, ssq, AF.Sqrt,
                                                 scale=1.0 / DIM, bias=eps128)
                            rr = ph1s.tile([128, 1], f32, tag="rr",
                                           name=f"rr{t}")
                            nc.vector.reciprocal(rr, rin)
                            diag = ph1s.tile([128, 128], bf16, tag="dg",
                                             name=f"dg{t}")
                            nc.vector.tensor_scalar_mul(diag, ident, rr)
                            xts.append(xt)
                            dgs.append(diag)
                        for c in range(KC):
                            pst = ph1ps.tile([128, 256], f32, tag="pt",
                                             name=f"pt{tg}_{half}_{c}")
                            for tt in range(2):
                                nc.tensor.matmul(pst[:, ts(tt, 128)],
                                                 xts[tt][:, ts(c, 128)],
                                                 dgs[tt],
                                                 start=True, stop=True)
                            with nc.allow_low_precision(reason="fp8 xn"):
                                nc.vector.tensor_scalar(
                                    out=xnT8[:, c,
                                             tg * 512 + half * 256 :
                                             tg * 512 + (half + 1) * 256],
                                    in0=pst,
                                    scalar1=mcol[:, c : c + 1],
                                    scalar2=scol[:, c : c + 1],
                                    op0=OP.mult, op1=OP.add,
                                )

                def emit_v_chunk(wvt, nv, t):
                    wvh_t, wvl_t = wvt
                    ps = shps.tile([128, 256], f32, tag="ps",
                                   name=f"v{nv}_{t}")
                    for cp in range(KCP):
                        nc.tensor.matmul(
                            ps, xnT8[:, 2 * cp : 2 * cp + 2, ts(t, 128)],
                            wvh_t[:, 2 * cp : 2 * cp + 2, :],
                            start=(cp == 0), stop=False, perf_mode=DR,
                        )
                    for cp in range(KCP):
                        nc.tensor.matmul(
                            ps, xnT8[:, 2 * cp : 2 * cp + 2, ts(t, 128)],
                            wvl_t[:, 2 * cp : 2 * cp + 2, :],
                            start=False, stop=(cp == KCP - 1),
                            perf_mode=DR,
                        )
                    with nc.allow_low_precision(reason="fp8 v"):
                        nc.scalar.activation(vS[:, t, ts(nv, 256)], ps,
                                             AF.Copy, scale=SV / SW)

                wvts = []
                for nv in range(4):
                    wvh_t = ph2v.tile([128, KC, 256], f8, tag=f"wvh{nv}",
                                      name=f"wvh{nv}")
                    nc.gpsimd.dma_start(out=wvh_t, in_=wvh[:, :, ts(nv, 256)])
                    wvl_t = ph2v.tile([128, KC, 256], f8l, tag=f"wvl{nv}",
                                      name=f"wvl{nv}")
                    nc.gpsimd.dma_start(out=wvl_t, in_=wvl[:, :, ts(nv, 256)])
                    wvts.append((wvh_t, wvl_t))
                for tg in range(4):
                    emit_ph1_group(tg)
                    # v for this group's tokens: xnT8[:, :, tg chunk] is ready
                    for nv in range(4):
                        for t in range(tg * 4, tg * 4 + 4):
                            emit_v_chunk(wvts[nv], nv, t)

            if stop_after == 'ph1':
                return

            # ---------- per-head pipeline: proj -> norm -> attention ------
            p25 = ph2stack.enter_context(tc.tile_pool(name="p25", bufs=2))
            p25r = ph2stack.enter_context(tc.tile_pool(name="p25r", bufs=2))
            at = ph2stack.enter_context(tc.tile_pool(name="at", bufs=2))
            Ep = ph2stack.enter_context(tc.tile_pool(name="Ep", bufs=1))
            E = Ep.tile([128, KC, 1024], f8)
            p25bc = ph2stack.enter_context(
                tc.tile_pool(name="p25bc", bufs=1, space="PSUM"))
            lps = ph2stack.enter_context(
                tc.tile_pool(name="lps", bufs=2, space="PSUM"))
            dps = ph2stack.enter_context(
                tc.tile_pool(name="dps", bufs=1, space="PSUM"))

            def emit_qk_w(m):
                wm = ph2w.tile([128, KC, 128], f8, tag="wqk", name=f"wm{m}")
                nc.gpsimd.dma_start(out=wm, in_=wqk[:, :, ts(m, 128)])
                return wm

            def emit_qk_group(wm, m, dst, nt):
                ps = shps.tile([128, 512], f32, tag="ps", name=f"qk{m}_{nt}")
                for cp in range(KCP):
                    nc.tensor.matmul(
                        ps, wm[:, 2 * cp : 2 * cp + 2, :],
                        xnT8[:, 2 * cp : 2 * cp + 2, ts(nt, 512)],
                        start=(cp == 0), stop=(cp == KCP - 1),
                        perf_mode=DR,
                    )
                with nc.allow_low_precision(reason="fp8 qk stage"):
                    nc.vector.tensor_scalar_mul(dst[:, ts(nt, 512)], ps, SST)

            def emit_norm_nt(h, src, sb, tagn, nt):
                # rv = 1/sqrt(colsum(src^2)*scale + bias), broadcast via PE
                sqc = p25.tile([128, 512], f8, tag="sq25",
                               name=f"sq{tagn}{h}_{nt}")
                with nc.allow_low_precision(reason="fp8 squares"):
                    nc.vector.tensor_mul(sqc, src[:, ts(nt, 512)],
                                         src[:, ts(nt, 512)])
                pb = p25bc.tile([128, 512], f32, tag="bc")
                nc.tensor.matmul(pb, onesq, sqc, start=True, stop=True)
                # rsqrt via exp(-0.5*ln(z)): stays in the exp table set,
                # avoiding Sqrt<->Exp activation-table thrash in the head loop
                lnz = p25r.tile([128, 512], bf16, tag="ln",
                                name=f"ln{tagn}{h}_{nt}")
                nc.scalar.activation(lnz, pb, AF.Ln,
                                     scale=sb[:, 0:1], bias=sb[:, 1:2])
                rvb = p25r.tile([128, 512], bf16, tag="rv",
                                name=f"rv{tagn}{h}_{nt}")
                nc.scalar.activation(rvb, lnz, AF.Exp, scale=-0.5)
                with nc.allow_low_precision(reason="fp8 qk norm"):
                    nc.vector.tensor_mul(src[:, ts(nt, 512)],
                                         src[:, ts(nt, 512)], rvb)

            def head_work(h):
                # thunks producing head h's normalized, d-split q/k
                qstg = stg.tile([128, N], f8, tag="qs", name=f"qstg{h}")
                kstg = stg.tile([128, N], f8, tag="ks", name=f"kstg{h}")
                th = []
                wq, wk = {}, {}
                th.append(lambda: wq.__setitem__(0, emit_qk_w(h)))
                for nt in range(4):
                    th.append(lambda nt=nt:
                              emit_qk_group(wq[0], h, qstg, nt))
                th.append(lambda: wk.__setitem__(0, emit_qk_w(H + h)))
                for nt in range(4):
                    th.append(lambda nt=nt:
                              emit_qk_group(wk[0], H + h, kstg, nt))
                for nt in range(4):
                    th.append(lambda nt=nt:
                              emit_norm_nt(h, qstg, qsb, "q", nt))
                for nt in range(4):
                    th.append(lambda nt=nt:
                              emit_norm_nt(h, kstg, ksb, "k", nt))

                def regroup():
                    for src, dstt in ((qstg, qT8s), (kstg, kT8s)):
                        nc.sync.dma_start(out=dstt[:, 0, h], in_=src[0:64])
                        nc.sync.dma_start(out=dstt[:, 1, h], in_=src[64:128])
                th.append(regroup)
                return th

            def emit_attention(h, pending):
                # interleave: pump next head's proj/norm between kc steps
                def pump(n=1):
                    for _ in range(n):
                        if pending:
                            pending.pop(0)()
                for qh in range(2):
                    q0 = qh * 1024
                    pd = dps.tile([128, 1024], f32, tag="dn",
                                  name=f"pd{h}_{qh}")
                    for kc in range(KC):
                        pl = lps.tile([128, 1024], f32, tag="lg")
                        for j in range(2):
                            nc.tensor.matmul(
                                pl[:, ts(j, 512)],
                                kT8s[:, :, h, ts(kc, 128)],
                                qT8s[:, :, h,
                                     q0 + j * 512 : q0 + (j + 1) * 512],
                                start=True, stop=True, perf_mode=DR,
                            )
                        nc.scalar.activation(E[:, kc, :], pl, AF.Exp,
                                             scale=1.0 / (SQK * SQK),
                                             bias=ln32n)
                        if kc % 2 == 1:
                            for qt in range(2):
                                nc.tensor.matmul(
                                    pd[:, ts(qt, 512)], ones8b,
                                    E[:, kc - 1 : kc + 1, ts(qt, 512)],
                                    start=(kc == 1), stop=(kc == KC - 1),
                                    perf_mode=DR,
                                )
                        pump(1)
                    for qt in range(2):
                        rb = at.tile([128, 512], bf16, tag="rb")
                        with nc.allow_low_precision(reason="bf16 denom"):
                            nc.vector.reciprocal(rb, pd[:, ts(qt, 512)])
                        po = shps.tile([128, 512], f32, tag="ps",
                                       name=f"po{h}_{qh}_{qt}")
                        for cp in range(KCP):
                            nc.tensor.matmul(
                                po, vS[:, 2 * cp : 2 * cp + 2, ts(h, 128)],
                                E[:, 2 * cp : 2 * cp + 2, ts(qt, 512)],
                                start=(cp == 0), stop=(cp == KCP - 1),
                                perf_mode=DR,
                            )
                        nc.vector.tensor_mul(
                            oT[:, h, q0 + qt * 512 : q0 + (qt + 1) * 512],
                            po, rb)
                        pump(1)

            pending = head_work(0)
            while pending:          # head 0 has no previous attention
                pending.pop(0)()
            for h in range(H):
                pending = head_work(h + 1) if h + 1 < H else []
                if stop_after == 'proj_only':
                    while pending:
                        pending.pop(0)()
                    continue
                emit_attention(h, pending)
                while pending:
                    pending.pop(0)()

        if stop_after in ('attn', 'proj_only'):
            return

        # ============ Phase 4: out = o @ (W_out + I) ===================
        with tc.tile_pool(name="ph4w", bufs=2) as ph4w, \
             tc.tile_pool(name="ph4o", bufs=3) as ph4o, \
             tc.tile_pool(name="ph4ps", bufs=4, space="PSUM") as ph4ps:
            for n_ in range(4):
                wop = ph4w.tile([128, H, 512], bf16, tag="wop",
                                name=f"wop{n_}")
                nc.sync.dma_start(out=wop, in_=wout[:, :, ts(n_, 512)])
                for t in range(TC):
                    ps = ph4ps.tile([128, 512], f32)
                    for oc in range(H):
                        nc.tensor.matmul(
                            ps, oT[:, oc, ts(t, 128)], wop[:, oc, :],
                            start=(oc == 0), stop=(oc == H - 1),
                        )
                    ot = ph4o.tile([128, 512], bf16, tag="ot")
                    nc.vector.tensor_copy(ot, ps)
                    nc.sync.dma_start(
                        out=out_p[ts(t, 128), ts(n_, 512)], in_=ot)


def _shard(inputs):
    import ml_dtypes
    BF = ml_dtypes.bfloat16
    F8 = ml_dtypes.float8_e4m3
    F8L = ml_dtypes.float8_e5m2

    x = np.ascontiguousarray(inputs["x"], dtype=np.float32)
    emb = np.asarray(inputs["emb"], dtype=np.float32)
    W_emb = np.asarray(inputs["W_emb"], dtype=np.float32)
    b_emb = np.asarray(inputs["b_emb"], dtype=np.float32)
    g_norm = np.asarray(inputs["g_norm"], dtype=np.float32)
    W_qkv = np.ascontiguousarray(inputs["W_qkv"], dtype=np.float32)
    g_q = np.asarray(inputs["g_q"], dtype=np.float32)
    g_k = np.asarray(inputs["g_k"], dtype=np.float32)
    W_out = np.ascontiguousarray(inputs["W_out"], dtype=np.float32)

    # tiny AdaLN conditioning projection done host-side (0.008% of FLOPs)
    ss = emb[:, 0, :] @ W_emb + b_emb          # [B, 2*DIM]
    scale, shift = ss[:, :DIM], ss[:, DIM:]
    mcol_b = (g_norm[None, :] * (1.0 + scale)).reshape(B, KC, 128)
    scol_b = shift.reshape(B, KC, 128)
    # per-partition Sqrt scale|bias for the QK-norm:
    # rv = 1/sqrt(colsum(q8^2)*s + b); q8 = SST*SW*q_true
    gq2 = (g_q * g_k).astype(np.float64) ** 2
    ssw = (SST * SW) ** 2
    qs_col = (D ** 0.5) / (SQK ** 2 * gq2 * D) / ssw
    qb_col = (D ** 0.5) / (SQK ** 2 * gq2) * EPS
    ks_col = np.full(D, (D ** 0.5) / (SQK ** 2 * D) / ssw)
    kb_col = np.full(D, (D ** 0.5) / (SQK ** 2) * EPS)
    qsb = np.stack([qs_col, qb_col], 1).astype(np.float32)
    ksb = np.stack([ks_col, kb_col], 1).astype(np.float32)

    in_maps = []
    for core in range(NCORES):
        b, g = core // HG, core % HG
        Wq = W_qkv[:, g * QK : (g + 1) * QK]
        Wk = W_qkv[:, DIM + g * QK : DIM + (g + 1) * QK]
        Wv = W_qkv[:, 2 * DIM + g * QK : 2 * DIM + (g + 1) * QK]
        Wqk8 = np.ascontiguousarray(
            np.concatenate([Wq, Wk], axis=1) * SW).astype(F8)
        Wvf = np.ascontiguousarray(Wv * SW)
        Wvhi8 = Wvf.astype(F8)
        Wvlo8 = (Wvf - Wvhi8.astype(np.float32)).astype(F8L)
        W_out_s = np.ascontiguousarray(W_out[g * QK : (g + 1) * QK, :]).copy()
        # fold residual: out = o_full @ (W_out + I); this core owns rows
        # g*QK..(g+1)*QK of the identity.
        idx = np.arange(QK)
        W_out_s[idx, g * QK + idx] += 1.0
        in_maps.append({
            "x_b": np.ascontiguousarray(x[b]).astype(BF),
            "mcol_in": np.ascontiguousarray(mcol_b[b].T),
            "scol_in": np.ascontiguousarray(scol_b[b].T),
            "qs_in": qsb,
            "ks_in": ksb,
            "Wqk8": Wqk8,
            "Wvhi8": Wvhi8,
            "Wvlo8": Wvlo8,
            "Wout_s": W_out_s.astype(BF),
        })
    return in_maps


def get_compiled():
    global _COMPILED
    if _COMPILED is None:
        _COMPILED = _build()
    return _COMPILED


def run_on_hw(inputs, trace=False):
    from concourse.bass_utils import run_bass_kernel_spmd

    nc = get_compiled()
    in_maps = _shard(inputs)
    res = run_bass_kernel_spmd(
        nc, in_maps, core_ids=list(range(NCORES)), trace=trace
    )
    out = np.empty((B, N, DIM), dtype=np.float32)
    for b in range(B):
        out[b] = (res.results[HG * b]["out_p"].astype(np.float32)
                  + res.results[HG * b + 1]["out_p"].astype(np.float32))
    return out, res


def kernel(**inputs) -> np.ndarray:
    out, _ = run_on_hw(inputs, trace=False)
    return out


# revision 17
# speedup vs baseline: 1.1217x; 1.1217x over previous
# Self-contained Trainium2 Bass kernel for nn_AttentionBlock (AdaLN + QK-norm
# attention), fp8-DoubleRow edition with per-head pipelining.
#
# Sharding: 8 cores = 4 batches (data parallel) x 2 head-groups of 8 heads
# (tensor parallel).  Each core computes, for its batch b and head group g:
#   xn^T  = (rmsnorm(x_b) * (1+scale) + shift)^T    fp8 e4m3 [dim, n]
#   v     = fp8 DR proj with e4m3-hi + e5m2-lo weight split  -> e4m3
#   per head h (pipelined so Act's exp overlaps the next head's proj):
#     q,k = fp8 DR proj -> staged e4m3 -> QK-rmsnorm -> e4m3 -> d-split DMA
#     E   = exp(logits/16 - ln32) e4m3; denom via fp8 DR ones-matmul
#     o^T = fp8 DR (v^T E) * recip(denom), bf16, kept in SBUF
#   out   = o @ (W_out + I) in bf16  -> bf16 partial, host sums the 2 groups
import numpy as np

B, N, DIM = 4, 2048, 2048
H_TOT, D = 16, 128
HG = 2                # head groups (tensor-parallel)
H = H_TOT // HG       # heads per core = 8
QK = H * D            # 1024 q (or k) columns per core
KC = DIM // 128       # 16 dim chunks
KCP = KC // 2         # 8 dim-chunk pairs (DoubleRow)
TC = N // 128         # 16 token chunks
EPS = 1e-6
NCORES = 8
SW = 32.0             # host fp8 scale on W_qkv
SST = 1.0 / 32.0      # q/k psum staging scale
SQK = 4.0             # q/k post-norm fp8 scale
SV = 4.0              # v fp8 scale
LN32 = float(np.log(32.0))

_COMPILED = None


def _build(stop_after=None):
    import concourse.bass as bass
    import concourse.bacc as bacc
    import concourse.tile as tile
    from concourse import mybir
    from concourse.masks import make_identity

    f32 = mybir.dt.float32
    bf16 = mybir.dt.bfloat16
    f8 = mybir.dt.float8e4
    f8l = mybir.dt.float8e5
    AF = mybir.ActivationFunctionType
    OP = mybir.AluOpType

    nc = bacc.Bacc(
        "TRN2", target_bir_lowering=False, debug=False, num_devices=NCORES
    )

    # ---- DRAM I/O -------------------------------------------------------
    x_b = nc.dram_tensor("x_b", [N, DIM], bf16, kind="ExternalInput").ap()
    mcol_in = nc.dram_tensor("mcol_in", [128, KC], f32, kind="ExternalInput").ap()
    scol_in = nc.dram_tensor("scol_in", [128, KC], f32, kind="ExternalInput").ap()
    qs_in = nc.dram_tensor("qs_in", [128, 2], f32, kind="ExternalInput").ap()
    ks_in = nc.dram_tensor("ks_in", [128, 2], f32, kind="ExternalInput").ap()
    Wqk8 = nc.dram_tensor("Wqk8", [DIM, 2 * QK], f8, kind="ExternalInput").ap()
    Wvhi8 = nc.dram_tensor("Wvhi8", [DIM, QK], f8, kind="ExternalInput").ap()
    Wvlo8 = nc.dram_tensor("Wvlo8", [DIM, QK], f8l, kind="ExternalInput").ap()
    Wout_s = nc.dram_tensor("Wout_s", [QK, DIM], bf16, kind="ExternalInput").ap()
    out_p = nc.dram_tensor("out_p", [N, DIM], bf16, kind="ExternalOutput").ap()

    with tile.TileContext(nc) as tc:
        _emit(nc, tc, bass, mybir, tile, make_identity, f32, bf16, f8, f8l,
              AF, OP, x_b, mcol_in, scol_in, qs_in, ks_in, Wqk8, Wvhi8,
              Wvlo8, Wout_s, out_p, stop_after)
    nc.compile()
    return nc


def _emit(nc, tc, bass, mybir, tile, make_identity, f32, bf16, f8, f8l,
          AF, OP, x_b, mcol_in, scol_in, qs_in, ks_in, Wqk8, Wvhi8,
          Wvlo8, Wout_s, out_p, stop_after=None):
    from contextlib import ExitStack

    ts = bass.ts
    DR = mybir.MatmulPerfMode.DoubleRow

    wqk = Wqk8.rearrange("(c p) n -> p c n", p=128)     # [128, KC, 2*QK]
    wvh = Wvhi8.rearrange("(c p) n -> p c n", p=128)    # [128, KC, QK]
    wvl = Wvlo8.rearrange("(c p) n -> p c n", p=128)
    wout = Wout_s.rearrange("(c p) n -> p c n", p=128)  # [128, H, DIM]

    with ExitStack() as ctx:
        consts = ctx.enter_context(tc.tile_pool(name="consts", bufs=1))

        ident = consts.tile([128, 128], bf16)
        make_identity(nc, ident)
        ones_col = consts.tile([128, 1], bf16)
        nc.vector.memset(ones_col, 1.0)
        ones8b = consts.tile([128, 2, 128], f8)   # DR denominator lhsT
        nc.vector.memset(ones8b, 4.0)             # folds E/32 and vS=4v scales
        onesq = consts.tile([128, 128], f8)       # colsum-broadcast lhsT
        nc.vector.memset(onesq, 1.0)

        mcol = consts.tile([128, KC], f32)          # g_norm*(1+scale) columns
        nc.sync.dma_start(out=mcol, in_=mcol_in)
        scol = consts.tile([128, KC], f32)          # shift columns
        nc.sync.dma_start(out=scol, in_=scol_in)
        qsb = consts.tile([128, 2], f32)            # q norm Sqrt scale|bias
        nc.sync.dma_start(out=qsb, in_=qs_in)
        ksb = consts.tile([128, 2], f32)            # k norm Sqrt scale|bias
        nc.sync.dma_start(out=ksb, in_=ks_in)
        eps128 = consts.tile([128, 1], f32)
        nc.vector.memset(eps128, EPS)
        eps1 = consts.tile([1, 1], f32)
        nc.vector.memset(eps1, EPS)
        ln32n = consts.tile([128, 1], f32)
        nc.vector.memset(ln32n, -LN32)

        # persistent across phases
        vS = consts.tile([128, TC, QK], f8)         # v * SV, token-major
        qT8s = consts.tile([64, 2, H, N], f8)       # d-split normalized q
        kT8s = consts.tile([64, 2, H, N], f8)
        oT = consts.tile([128, H, N], bf16)         # o^T, stays in SBUF
        if stop_after == 'ph0':
            return

        with ExitStack() as ph2stack:
            xnp = ph2stack.enter_context(tc.tile_pool(name="xnp", bufs=1))
            xnT8 = xnp.tile([128, KC, N], f8)
            stg = ph2stack.enter_context(tc.tile_pool(name="stg", bufs=2))
            ph2w = ph2stack.enter_context(tc.tile_pool(name="ph2w", bufs=3))
            # shared psum for qk-proj ps / v-proj ps / attention po
            shps = ph2stack.enter_context(
                tc.tile_pool(name="shps", bufs=1, space="PSUM"))

            # ---------- phase 1 + v-projection (before the head loop) ----
            with ExitStack() as innerA:
                ph1 = innerA.enter_context(tc.tile_pool(name="ph1", bufs=2))
                ph1s = innerA.enter_context(tc.tile_pool(name="ph1s", bufs=4))
                ph1sq = innerA.enter_context(tc.tile_pool(name="ph1sq", bufs=1))
                ph2v = innerA.enter_context(tc.tile_pool(name="ph2v", bufs=1))
                ph1ps = innerA.enter_context(
                    tc.tile_pool(name="ph1ps", bufs=2, space="PSUM"))

                def emit_ph1_group(tg):
                    for half in range(2):
                        xts, dgs = [], []
                        for tt in range(2):
                            t = tg * 4 + half * 2 + tt
                            xt = ph1.tile([128, DIM], bf16, tag="xt",
                                          name=f"xt{t}")
                            nc.sync.dma_start(out=xt, in_=x_b[ts(t, 128), :])
                            sq = ph1sq.tile([128, DIM], bf16, tag="sq",
                                            name=f"sq{t}")
                            ssq = ph1s.tile([128, 1], f32, tag="ssq",
                                            name=f"ssq{t}")
                            nc.scalar.activation(sq, xt, AF.Square,
                                                 accum_out=ssq)
                            rin = ph1s.tile([128, 1], f32, tag="rin",
                                            name=f"ri{t}")
                            nc.scalar.activation(rin, ssq, AF.Sqrt,
                                                 scale=1.0 / DIM, bias=eps128)
                            rr = ph1s.tile([128, 1], f32, tag="rr",
                                           name=f"rr{t}")
                            nc.vector.reciprocal(rr, rin)
                            diag = ph1s.tile([128, 128], bf16, tag="dg",
                                             name=f"dg{t}")
                            nc.vector.tensor_scalar_mul(diag, ident, rr)
                            xts.append(xt)
                            dgs.append(diag)
                        for c in range(KC):
                            pst = ph1ps.tile([128, 256], f32, tag="pt",
                                             name=f"pt{tg}_{half}_{c}")
                            for tt in range(2):
                                nc.tensor.matmul(pst[:, ts(tt, 128)],
                                                 xts[tt][:, ts(c, 128)],
                                                 dgs[tt],
                                                 start=True, stop=True)
                            with nc.allow_low_precision(reason="fp8 xn"):
                                nc.vector.tensor_scalar(
                                    out=xnT8[:, c,
                                             tg * 512 + half * 256 :
                                             tg * 512 + (half + 1) * 256],
                                    in0=pst,
                                    scalar1=mcol[:, c : c + 1],
                                    scalar2=scol[:, c : c + 1],
                                    op0=OP.mult, op1=OP.add,
                                )

                def emit_v_chunk(wvt, nv, t):
                    wvh_t, wvl_t = wvt
                    ps = shps.tile([128, 256], f32, tag="ps",
                                   name=f"v{nv}_{t}")
                    for cp in range(KCP):
                        nc.tensor.matmul(
                            ps, xnT8[:, 2 * cp : 2 * cp + 2, ts(t, 128)],
                            wvh_t[:, 2 * cp : 2 * cp + 2, :],
                            start=(cp == 0), stop=False, perf_mode=DR,
                        )
                    for cp in range(KCP):
                        nc.tensor.matmul(
                            ps, xnT8[:, 2 * cp : 2 * cp + 2, ts(t, 128)],
                            wvl_t[:, 2 * cp : 2 * cp + 2, :],
                            start=False, stop=(cp == KCP - 1),
                            perf_mode=DR,
                        )
                    with nc.allow_low_precision(reason="fp8 v"):
                        nc.scalar.activation(vS[:, t, ts(nv, 256)], ps,
                                             AF.Copy, scale=SV / SW)

                wvts = []
                for nv in range(4):
                    wvh_t = ph2v.tile([128, KC, 256], f8, tag=f"wvh{nv}",
                                      name=f"wvh{nv}")
                    nc.gpsimd.dma_start(out=wvh_t, in_=wvh[:, :, ts(nv, 256)])
                    wvl_t = ph2v.tile([128, KC, 256], f8l, tag=f"wvl{nv}",
                                      name=f"wvl{nv}")
                    nc.gpsimd.dma_start(out=wvl_t, in_=wvl[:, :, ts(nv, 256)])
                    wvts.append((wvh_t, wvl_t))
                for tg in range(4):
                    emit_ph1_group(tg)
                    # v for this group's tokens: xnT8[:, :, tg chunk] is ready
                    for nv in range(4):
                        for t in range(tg * 4, tg * 4 + 4):
                            emit_v_chunk(wvts[nv], nv, t)

            if stop_after == 'ph1':
                return

            # ---------- per-head pipeline: proj -> norm -> attention ------
            p25 = ph2stack.enter_context(tc.tile_pool(name="p25", bufs=2))
            p25r = ph2stack.enter_context(tc.tile_pool(name="p25r", bufs=2))
            at = ph2stack.enter_context(tc.tile_pool(name="at", bufs=2))
            Ep = ph2stack.enter_context(tc.tile_pool(name="Ep", bufs=1))
            E = Ep.tile([128, KC, 1024], f8)
            p25bc = ph2stack.enter_context(
                tc.tile_pool(name="p25bc", bufs=1, space="PSUM"))
            lps = ph2stack.enter_context(
                tc.tile_pool(name="lps", bufs=2, space="PSUM"))
            dps = ph2stack.enter_context(
                tc.tile_pool(name="dps", bufs=1, space="PSUM"))

            def emit_qk_w(m):
                wm = ph2w.tile([128, KC, 128], f8, tag="wqk", name=f"wm{m}")
                nc.gpsimd.dma_start(out=wm, in_=wqk[:, :, ts(m, 128)])
                return wm

            def emit_qk_group(wm, m, dst, nt):
                ps = shps.tile([128, 512], f32, tag="ps", name=f"qk{m}_{nt}")
                for cp in range(KCP):
                    nc.tensor.matmul(
                        ps, wm[:, 2 * cp : 2 * cp + 2, :],
                        xnT8[:, 2 * cp : 2 * cp + 2, ts(nt, 512)],
                        start=(cp == 0), stop=(cp == KCP - 1),
                        perf_mode=DR,
                    )
                with nc.allow_low_precision(reason="fp8 qk stage"):
                    nc.vector.tensor_scalar_mul(dst[:, ts(nt, 512)], ps, SST)

            def emit_norm_nt(h, src, sb, tagn, nt):
                # rv = 1/sqrt(colsum(src^2)*scale + bias), broadcast via PE
                sqc = p25.tile([128, 512], f8, tag="sq25",
                               name=f"sq{tagn}{h}_{nt}")
                with nc.allow_low_precision(reason="fp8 squares"):
                    nc.vector.tensor_mul(sqc, src[:, ts(nt, 512)],
                                         src[:, ts(nt, 512)])
                pb = p25bc.tile([128, 512], f32, tag="bc")
                nc.tensor.matmul(pb, onesq, sqc, start=True, stop=True)
                rvb = p25r.tile([128, 512], bf16, tag="rv",
                                name=f"rv{tagn}{h}_{nt}")
                nc.scalar.activation(rvb, pb, AF.Sqrt,
                                     scale=sb[:, 0:1], bias=sb[:, 1:2])
                with nc.allow_low_precision(reason="fp8 qk norm"):
                    nc.vector.reciprocal(rvb, rvb)
                    nc.vector.tensor_mul(src[:, ts(nt, 512)],
                                         src[:, ts(nt, 512)], rvb)

            def head_proj_thunks(h, qstg, kstg):
                # table-neutral work (matmuls, DVE) pumped into the exp stream
                th = []
                wq, wk = {}, {}
                th.append(lambda: wq.__setitem__(0, emit_qk_w(h)))
                for nt in range(4):
                    th.append(lambda nt=nt:
                              emit_qk_group(wq[0], h, qstg, nt))
                th.append(lambda: wk.__setitem__(0, emit_qk_w(H + h)))
                for nt in range(4):
                    th.append(lambda nt=nt:
                              emit_qk_group(wk[0], H + h, kstg, nt))
                return th

            def head_norm_block(h, qstg, kstg):
                # Sqrt burst: 2 activation-table switches per head, not 16
                for nt in range(4):
                    emit_norm_nt(h, qstg, qsb, "q", nt)
                for nt in range(4):
                    emit_norm_nt(h, kstg, ksb, "k", nt)
                for src, dstt in ((qstg, qT8s), (kstg, kT8s)):
                    nc.sync.dma_start(out=dstt[:, 0, h], in_=src[0:64])
                    nc.sync.dma_start(out=dstt[:, 1, h], in_=src[64:128])

            def emit_attention(h, pending):
                # interleave: pump next head's proj/norm between kc steps
                def pump(n=1):
                    for _ in range(n):
                        if pending:
                            pending.pop(0)()
                for qh in range(2):
                    q0 = qh * 1024
                    pd = dps.tile([128, 1024], f32, tag="dn",
                                  name=f"pd{h}_{qh}")
                    for kc in range(KC):
                        pl = lps.tile([128, 1024], f32, tag="lg")
                        for j in range(2):
                            nc.tensor.matmul(
                                pl[:, ts(j, 512)],
                                kT8s[:, :, h, ts(kc, 128)],
                                qT8s[:, :, h,
                                     q0 + j * 512 : q0 + (j + 1) * 512],
                                start=True, stop=True, perf_mode=DR,
                            )
                        nc.scalar.activation(E[:, kc, :], pl, AF.Exp,
                                             scale=1.0 / (SQK * SQK),
                                             bias=ln32n)
                        if kc % 2 == 1:
                            for qt in range(2):
                                nc.tensor.matmul(
                                    pd[:, ts(qt, 512)], ones8b,
                                    E[:, kc - 1 : kc + 1, ts(qt, 512)],
                                    start=(kc == 1), stop=(kc == KC - 1),
                                    perf_mode=DR,
                                )
                        pump(1)
                    for qt in range(2):
                        rb = at.tile([128, 512], bf16, tag="rb")
                        with nc.allow_low_precision(reason="bf16 denom"):
                            nc.vector.reciprocal(rb, pd[:, ts(qt, 512)])
                        po = shps.tile([128, 512], f32, tag="ps",
                                       name=f"po{h}_{qh}_{qt}")
                        for cp in range(KCP):
                            nc.tensor.matmul(
                                po, vS[:, 2 * cp : 2 * cp + 2, ts(h, 128)],
                                E[:, 2 * cp : 2 * cp + 2, ts(qt, 512)],
                                start=(cp == 0), stop=(cp == KCP - 1),
                                perf_mode=DR,
                            )
                        nc.vector.tensor_mul(
                            oT[:, h, q0 + qt * 512 : q0 + (qt + 1) * 512],
                            po, rb)
                        pump(1)

            stgs = {}
            for hh in range(H):
                stgs[hh] = (stg.tile([128, N], f8, tag="qs", name=f"qstg{hh}"),
                            stg.tile([128, N], f8, tag="ks", name=f"kstg{hh}"))
            for t in head_proj_thunks(0, *stgs[0]):
                t()
            head_norm_block(0, *stgs[0])
            for h in range(H):
                pending = (head_proj_thunks(h + 1, *stgs[h + 1])
                           if h + 1 < H else [])
                if stop_after == 'proj_only':
                    while pending:
                        pending.pop(0)()
                    continue
                emit_attention(h, pending)
                while pending:
                    pending.pop(0)()
                if h + 1 < H:
                    head_norm_block(h + 1, *stgs[h + 1])

        if stop_after in ('attn', 'proj_only'):
            return

        # ============ Phase 4: out = o @ (W_out + I) ===================
        with tc.tile_pool(name="ph4w", bufs=2) as ph4w, \
             tc.tile_pool(name="ph4o", bufs=3) as ph4o, \
             tc.tile_pool(name="ph4ps", bufs=4, space="PSUM") as ph4ps:
            for n_ in range(4):
                wop = ph4w.tile([128, H, 512], bf16, tag="wop",
                                name=f"wop{n_}")
                nc.sync.dma_start(out=wop, in_=wout[:, :, ts(n_, 512)])
                for t in range(TC):
                    ps = ph4ps.tile([128, 512], f32)
                    for oc in range(H):
                        nc.tensor.matmul(
                            ps, oT[:, oc, ts(t, 128)], wop[:, oc, :],
                            start=(oc == 0), stop=(oc == H - 1),
                        )
                    ot = ph4o.tile([128, 512], bf16, tag="ot")
                    nc.vector.tensor_copy(ot, ps)
                    nc.sync.dma_start(
                        out=out_p[ts(t, 128), ts(n_, 512)], in_=ot)


def _shard(inputs):
    import ml_dtypes
    BF = ml_dtypes.bfloat16
    F8 = ml_dtypes.float8_e4m3
    F8L = ml_dtypes.float8_e5m2

    x = np.ascontiguousarray(inputs["x"], dtype=np.float32)
    emb = np.asarray(inputs["emb"], dtype=np.float32)
    W_emb = np.asarray(inputs["W_emb"], dtype=np.float32)
    b_emb = np.asarray(inputs["b_emb"], dtype=np.float32)
    g_norm = np.asarray(inputs["g_norm"], dtype=np.float32)
    W_qkv = np.ascontiguousarray(inputs["W_qkv"], dtype=np.float32)
    g_q = np.asarray(inputs["g_q"], dtype=np.float32)
    g_k = np.asarray(inputs["g_k"], dtype=np.float32)
    W_out = np.ascontiguousarray(inputs["W_out"], dtype=np.float32)

    # tiny AdaLN conditioning projection done host-side (0.008% of FLOPs)
    ss = emb[:, 0, :] @ W_emb + b_emb          # [B, 2*DIM]
    scale, shift = ss[:, :DIM], ss[:, DIM:]
    mcol_b = (g_norm[None, :] * (1.0 + scale)).reshape(B, KC, 128)
    scol_b = shift.reshape(B, KC, 128)
    # per-partition Sqrt scale|bias for the QK-norm:
    # rv = 1/sqrt(colsum(q8^2)*s + b); q8 = SST*SW*q_true
    gq2 = (g_q * g_k).astype(np.float64) ** 2
    ssw = (SST * SW) ** 2
    qs_col = (D ** 0.5) / (SQK ** 2 * gq2 * D) / ssw
    qb_col = (D ** 0.5) / (SQK ** 2 * gq2) * EPS
    ks_col = np.full(D, (D ** 0.5) / (SQK ** 2 * D) / ssw)
    kb_col = np.full(D, (D ** 0.5) / (SQK ** 2) * EPS)
    qsb = np.stack([qs_col, qb_col], 1).astype(np.float32)
    ksb = np.stack([ks_col, kb_col], 1).astype(np.float32)

    in_maps = []
    for core in range(NCORES):
        b, g = core // HG, core % HG
        Wq = W_qkv[:, g * QK : (g + 1) * QK]
        Wk = W_qkv[:, DIM + g * QK : DIM + (g + 1) * QK]
        Wv = W_qkv[:, 2 * DIM + g * QK : 2 * DIM + (g + 1) * QK]
        Wqk8 = np.ascontiguousarray(
            np.concatenate([Wq, Wk], axis=1) * SW).astype(F8)
        Wvf = np.ascontiguousarray(Wv * SW)
        Wvhi8 = Wvf.astype(F8)
        Wvlo8 = (Wvf - Wvhi8.astype(np.float32)).astype(F8L)
        W_out_s = np.ascontiguousarray(W_out[g * QK : (g + 1) * QK, :]).copy()
        # fold residual: out = o_full @ (W_out + I); this core owns rows
        # g*QK..(g+1)*QK of the identity.
        idx = np.arange(QK)
        W_out_s[idx, g * QK + idx] += 1.0
        in_maps.append({
            "x_b": np.ascontiguousarray(x[b]).astype(BF),
            "mcol_in": np.ascontiguousarray(mcol_b[b].T),
            "scol_in": np.ascontiguousarray(scol_b[b].T),
            "qs_in": qsb,
            "ks_in": ksb,
            "Wqk8": Wqk8,
            "Wvhi8": Wvhi8,
            "Wvlo8": Wvlo8,
            "Wout_s": W_out_s.astype(BF),
        })
    return in_maps


def get_compiled():
    global _COMPILED
    if _COMPILED is None:
        _COMPILED = _build()
    return _COMPILED


def run_on_hw(inputs, trace=False):
    from concourse.bass_utils import run_bass_kernel_spmd

    nc = get_compiled()
    in_maps = _shard(inputs)
    res = run_bass_kernel_spmd(
        nc, in_maps, core_ids=list(range(NCORES)), trace=trace
    )
    out = np.empty((B, N, DIM), dtype=np.float32)
    for b in range(B):
        out[b] = (res.results[HG * b]["out_p"].astype(np.float32)
                  + res.results[HG * b + 1]["out_p"].astype(np.float32))
    return out, res


def kernel(**inputs) -> np.ndarray:
    out, _ = run_on_hw(inputs, trace=False)
    return out
